# revision 1
# baseline (speedup 1.0000x reference)
"""EquiConv (DeepH-E3) Trainium2 kernel — 8-core data-parallel over edges.

Strategy (channel-major on device):
  - Host folds all per-channel weights/constants into matmul weight
    matrices, shards edges across 8 cores, pads to a multiple of 512 and
    transposes edge tensors to channel-major [C, E].
  - Per-edge scalars (x2s, x2v_i) are shipped pre-replicated across 128
    partitions (host-side layout transform), so device tiles need only
    DVE multiplies to pre-scale activations; all tensor-product paths
    become accumulating float32r matmuls into PSUM, the Gate uses
    Silu/Tanh (one ACT table set; sigmoid(g)*w computed via a fused
    scalar_tensor_tensor from tanh(g/2)), and the e3ElementWise multiply
    is fused into the output elementwise ops. DMA issue is spread over
    the Sync and GpSimd descriptor queues.
  - Host transposes the [320, E] channel-major output back.

Self-contained: hardcodes shapes from the problem spec; no file reads.
"""
import os
import sys

import numpy as np

# ---------------------------------------------------------------- constants
E_FULL = 200000
N_CORES = 8
E_CORE = E_FULL // N_CORES      # 25000
NT = 512                        # edges per tile
T_TILES = 49                    # tiles per core
E_PAD = NT * T_TILES            # 25088
MUL_S = 128
MUL_V = 64

INV_S = 1.0 / np.sqrt(MUL_S)
INV_V = 1.0 / np.sqrt(MUL_V)
SQ2 = 1.0 / np.sqrt(2.0)
SQ3 = 1.0 / np.sqrt(3.0)

_REPO_CANDIDATES = (
    "/opt/trn_rl_repo",
    "/root/.axon_site/_ro/trn_rl_repo",
)


def _ensure_repo_on_path():
    try:
        import concourse.bass  # noqa: F401
        return
    except ImportError:
        pass
    for p in _REPO_CANDIDATES:
        if os.path.isdir(p) and p not in sys.path:
            sys.path.insert(0, p)
    import concourse.bass  # noqa: F401


_CACHE = {}


def _build_nc():
    """Build + compile the per-core Bass program (cached)."""
    if "nc" in _CACHE:
        return _CACHE["nc"]
    _ensure_repo_on_path()
    import concourse.mybir as mybir
    import concourse.tile as tile
    from concourse import bacc

    F32 = mybir.dt.float32
    F32R = mybir.dt.float32r
    MULT = mybir.AluOpType.mult
    ADD = mybir.AluOpType.add
    AF = mybir.ActivationFunctionType

    nc = bacc.Bacc(trn_type="TRN2", target_bir_lowering=False, debug=False,
                   num_devices=N_CORES)

    # DRAM inputs (per-core shard, channel-major) --------------------------
    d_x1s = nc.dram_tensor("x1s_t", [128, E_PAD], F32R, kind="ExternalInput")
    d_x1v = nc.dram_tensor("x1v_t", [192, E_PAD], F32R, kind="ExternalInput")
    d_rs = nc.dram_tensor("r_s", [128, E_PAD], F32R, kind="ExternalInput")
    d_rv0 = nc.dram_tensor("r_v0", [128, E_PAD], F32R, kind="ExternalInput")
    d_rv1 = nc.dram_tensor("r_v1", [128, E_PAD], F32R, kind="ExternalInput")
    d_rv2 = nc.dram_tensor("r_v2", [128, E_PAD], F32R, kind="ExternalInput")
    d_rv01 = nc.dram_tensor("r_v01", [128, E_PAD], F32R,
                            kind="ExternalInput")
    d_fw = nc.dram_tensor("fw_t", [128, E_PAD], F32R, kind="ExternalInput")
    # folded weights ([K, M] layouts, ready as lhsT)
    d_wa0 = nc.dram_tensor("wa0", [128, 128], F32R, kind="ExternalInput")
    d_wa1 = nc.dram_tensor("wa1", [128, 64], F32R, kind="ExternalInput")
    d_wp2 = nc.dram_tensor("wp2", [128, 64], F32R, kind="ExternalInput")
    d_wb4s = nc.dram_tensor("wb4s", [128, 128], F32R, kind="ExternalInput")
    d_wb4b = nc.dram_tensor("wb4b", [64, 128], F32R, kind="ExternalInput")
    d_wb5s = nc.dram_tensor("wb5s", [128, 64], F32R, kind="ExternalInput")
    d_wb5b = nc.dram_tensor("wb5b", [64, 64], F32R, kind="ExternalInput")
    d_wc = nc.dram_tensor("wc", [64, 64], F32R, kind="ExternalInput")
    d_fc0 = nc.dram_tensor("fc0", [128, 64], F32R, kind="ExternalInput")
    d_fc1 = nc.dram_tensor("fc1", [64, 64], F32R, kind="ExternalInput")
    d_fc2a = nc.dram_tensor("fc2a", [64, 128], F32R, kind="ExternalInput")
    d_fc2b = nc.dram_tensor("fc2b", [64, 64], F32R, kind="ExternalInput")
    d_b0 = nc.dram_tensor("b0c", [64, 1], F32, kind="ExternalInput")
    d_b1 = nc.dram_tensor("b1c", [64, 1], F32, kind="ExternalInput")
    d_b2a = nc.dram_tensor("b2a", [128, 1], F32, kind="ExternalInput")
    d_b2b = nc.dram_tensor("b2v", [64, 1], F32, kind="ExternalInput")

    d_out = nc.dram_tensor("out_t", [320, E_PAD], F32, kind="ExternalOutput")

    with tile.TileContext(nc) as tc:
        with tc.tile_pool(name="const", bufs=1) as cp, \
             tc.tile_pool(name="io", bufs=4) as io, \
             tc.tile_pool(name="work", bufs=2) as wk, \
             tc.tile_pool(name="ps", bufs=1, space="PSUM") as ps:

            # constants into SBUF once
            def const(d, shape, dtype=F32R, name=None):
                t = cp.tile(shape, dtype, name=name or d.name + "_sb")
                nc.sync.dma_start(t, d.ap())
                return t

            w_wa0 = const(d_wa0, [128, 128])
            w_wa1 = const(d_wa1, [128, 64])
            w_wp2 = const(d_wp2, [128, 64])
            w_wb4s = const(d_wb4s, [128, 128])
            w_wb4b = const(d_wb4b, [64, 128])
            w_wb5s = const(d_wb5s, [128, 64])
            w_wb5b = const(d_wb5b, [64, 64])
            w_wc = const(d_wc, [64, 64])
            w_fc0 = const(d_fc0, [128, 64])
            w_fc1 = const(d_fc1, [64, 64])
            w_fc2a = const(d_fc2a, [64, 128])
            w_fc2b = const(d_fc2b, [64, 64])
            c_b0 = const(d_b0, [64, 1], F32)
            c_b1 = const(d_b1, [64, 1], F32)
            c_b2a = const(d_b2a, [128, 1], F32)
            c_b2b = const(d_b2b, [64, 1], F32)
            # WC copy living at partitions 64-127 for the row-offset matmul
            w_wc_f = cp.tile([128, 64], F32R)
            w_wc_hi = w_wc_f[64:128, :]
            nc.sync.dma_start(w_wc_hi, d_wc.ap())

            for t in range(T_TILES):
                sl = slice(t * NT, (t + 1) * NT)

                # ---- loads -------------------------------------------
                x1s = io.tile([128, NT], F32R)
                nc.sync.dma_start(x1s, d_x1s.ap()[:, sl])
                x1va = io.tile([128, NT], F32R)
                nc.sync.dma_start(x1va, d_x1v.ap()[0:128, sl])
                x1vc2 = io.tile([64, NT], F32R)
                nc.sync.dma_start(x1vc2, d_x1v.ap()[128:192, sl])
                rep_s = io.tile([128, NT], F32R)
                nc.gpsimd.dma_start(rep_s, d_rs.ap()[:, sl])
                rep_v0 = io.tile([128, NT], F32R)
                nc.gpsimd.dma_start(rep_v0, d_rv0.ap()[:, sl])
                rep_v1 = io.tile([128, NT], F32R)
                nc.gpsimd.dma_start(rep_v1, d_rv1.ap()[:, sl])
                rep_v2 = io.tile([128, NT], F32R)
                nc.gpsimd.dma_start(rep_v2, d_rv2.ap()[:, sl])
                rep_v01 = io.tile([128, NT], F32R)
                nc.gpsimd.dma_start(rep_v01, d_rv01.ap()[:, sl])
                fwt = io.tile([128, NT], F32R)
                nc.gpsimd.dma_start(fwt, d_fw.ap()[:, sl])

                # ---- radial MLP --------------------------------------
                h1 = ps.tile([64, NT], F32, tag="mlp")
                nc.tensor.matmul(h1, w_fc0, fwt, start=True, stop=True)
                h1s = wk.tile([64, NT], F32R)
                nc.scalar.activation(h1s, h1, AF.Silu, bias=c_b0)
                h2 = ps.tile([64, NT], F32, tag="mlp")
                nc.tensor.matmul(h2, w_fc1, h1s, start=True, stop=True)
                h2s = wk.tile([64, NT], F32R)
                nc.scalar.activation(h2s, h2, AF.Silu, bias=c_b1)
                wwa = ps.tile([128, NT], F32, tag="mlp")
                nc.tensor.matmul(wwa, w_fc2a, h2s, start=True, stop=True)
                wwa_s = wk.tile([128, NT], F32)
                nc.scalar.activation(wwa_s, wwa, AF.Identity, bias=c_b2a)
                wwb = ps.tile([64, NT], F32, tag="mlp")
                nc.tensor.matmul(wwb, w_fc2b, h2s, start=True, stop=True)
                wwb_s = wk.tile([64, NT], F32)
                # 0.5*(wwb + b2v): folds the sigmoid's 0.5 factor
                nc.scalar.activation(wwb_s, wwb, AF.Identity, bias=c_b2b,
                                     scale=0.5)

                # ---- prescales (ordered to unblock PE early) ---------
                x1s_s = wk.tile([128, NT], F32R, bufs=3)
                nc.vector.tensor_tensor(x1s_s, x1s, rep_s, MULT)
                x1s_v0 = wk.tile([128, NT], F32R, bufs=3)
                nc.vector.tensor_tensor(x1s_v0, x1s, rep_v0, MULT)
                x1s_v1 = wk.tile([128, NT], F32R, bufs=3)
                nc.vector.tensor_tensor(x1s_v1, x1s, rep_v1, MULT)
                x1s_v2 = wk.tile([128, NT], F32R, bufs=3)
                nc.vector.tensor_tensor(x1s_v2, x1s, rep_v2, MULT)
                xv_s01 = wk.tile([128, NT], F32R, bufs=3)
                nc.vector.tensor_tensor(xv_s01, x1va, rep_s, MULT)
                xv_p01 = wk.tile([128, NT], F32R, bufs=3)
                nc.vector.tensor_tensor(xv_p01, x1va, rep_v01, MULT)
                xv_s2 = wk.tile([64, NT], F32R, bufs=3)
                nc.vector.tensor_tensor(xv_s2, x1vc2, rep_s[0:64, :], MULT)
                xv_p2 = wk.tile([64, NT], F32R, bufs=3)
                nc.vector.tensor_tensor(xv_p2, x1vc2, rep_v2[0:64, :], MULT)

                # ---- tensor-product matmuls (consumption order) ------
                scal = ps.tile([128, NT], F32, tag="scal", bufs=2)
                gate = ps.tile([64, NT], F32, tag="gate", bufs=2)
                vec0 = ps.tile([64, NT], F32, tag="vec0")
                vec1 = ps.tile([64, NT], F32, tag="vec1")
                vec2 = ps.tile([64, NT], F32, tag="vec2")
                nc.tensor.matmul(scal, w_wa0, x1s_s, start=True, stop=False)
                nc.tensor.matmul(gate, w_wa1, x1s_s, start=True, stop=False)
                nc.tensor.matmul(vec0, w_wp2, x1s_v0, start=True, stop=False)
                nc.tensor.matmul(vec1, w_wp2, x1s_v1, start=True, stop=False)
                nc.tensor.matmul(vec2, w_wp2, x1s_v2, start=True, stop=False)
                nc.tensor.matmul(vec0, w_wc, xv_s01[0:64, :],
                                 start=False, stop=True)
                nc.tensor.matmul(vec1, w_wc_hi, xv_s01[64:128, :],
                                 start=False, stop=True,
                                 tile_position=(64, 0))
                nc.tensor.matmul(scal, w_wb4s, xv_p01, start=False, stop=False)
                nc.tensor.matmul(gate, w_wb5s, xv_p01, start=False, stop=False)
                nc.tensor.matmul(vec2, w_wc, xv_s2, start=False, stop=True)
                nc.tensor.matmul(scal, w_wb4b, xv_p2, start=False, stop=True)
                nc.tensor.matmul(gate, w_wb5b, xv_p2, start=False, stop=True)

                # ---- gate + e3ElementWise ----------------------------
                sc_silu = wk.tile([128, NT], F32)
                nc.scalar.activation(sc_silu, scal, AF.Silu)
                tgate = wk.tile([64, NT], F32)
                nc.scalar.activation(tgate, gate, AF.Tanh, scale=0.5)
                # sgw = (tanh(g/2)+1) * 0.5*(w_vec+b) = sigmoid(g)*w_vec
                sgw = wk.tile([64, NT], F32)
                nc.vector.scalar_tensor_tensor(sgw, tgate, 1.0, wwb_s,
                                               ADD, MULT)

                out_s = wk.tile([128, NT], F32)
                nc.vector.tensor_tensor(out_s, sc_silu, wwa_s, MULT)
                out0 = wk.tile([64, NT], F32)
                nc.vector.tensor_tensor(out0, vec0, sgw, MULT)
                out1 = wk.tile([64, NT], F32)
                nc.vector.tensor_tensor(out1, vec1, sgw, MULT)
                out2 = wk.tile([64, NT], F32)
                nc.vector.tensor_tensor(out2, vec2, sgw, MULT)

                # ---- stores ------------------------------------------
                nc.sync.dma_start(d_out.ap()[0:128, sl], out_s)
                nc.sync.dma_start(d_out.ap()[128:192, sl], out0)
                nc.sync.dma_start(d_out.ap()[192:256, sl], out1)
                nc.sync.dma_start(d_out.ap()[256:320, sl], out2)

    nc.compile()
    _CACHE["nc"] = nc
    return nc


def _fold_weights(inp):
    """Fold per-channel weights + constants into matmul matrices."""
    f = lambda k: np.asarray(inp[k], dtype=np.float32)
    w0f = f("w1_p0") * f("w2_p0")[None, :] * (INV_S * SQ2)
    w1f = f("w1_p1") * f("w2_p1")[None, :] * (INV_S * SQ2)
    w2f = f("w1_p2") * f("w2_p2")[None, :] * (INV_S * SQ2)
    w3f = f("w1_p3") * f("w2_p3")[None, :] * (INV_V * SQ2)
    w4f = f("w1_p4") * f("w2_p4")[None, :] * (INV_V * SQ3 * SQ2)
    w5f = f("w1_p5") * f("w2_p5")[None, :] * (INV_V * SQ3 * SQ2)
    fc2 = f("fc_w2")
    b2 = f("fc_b2")
    c = np.ascontiguousarray
    return {
        "wa0": c(w0f),
        "wa1": c(w1f),
        "wp2": c(w2f),
        "wb4s": c(np.concatenate([w4f, w4f], axis=0)),
        "wb4b": c(w4f),
        "wb5s": c(np.concatenate([w5f, w5f], axis=0)),
        "wb5b": c(w5f),
        "wc": c(w3f),
        "fc0": c(f("fc_w0")),
        "fc1": c(f("fc_w1")),
        "fc2a": c(fc2[:, :128]),
        "fc2b": c(fc2[:, 128:]),
        "b0c": c(f("fc_b0")[:, None]),
        "b1c": c(f("fc_b1")[:, None]),
        "b2a": c(b2[:128, None]),
        "b2v": c(b2[128:, None]),

    }


def _shard_inputs(inp):
    """Per-core channel-major shards (padded to E_PAD edges)."""
    fea_in1 = np.asarray(inp["fea_in1"], dtype=np.float32)
    fea_in2 = np.asarray(inp["fea_in2"], dtype=np.float32)
    fea_w = np.asarray(inp["fea_weight"], dtype=np.float32)
    shards = []
    for c in range(N_CORES):
        s = slice(c * E_CORE, (c + 1) * E_CORE)
        x1 = fea_in1[s]
        x2 = fea_in2[s]
        fw = fea_w[s]
        x1s_t = np.zeros((128, E_PAD), np.float32)
        x1s_t[:, :E_CORE] = x1[:, :128].T
        x1v_t = np.zeros((192, E_PAD), np.float32)
        x1v_t[:, :E_CORE] = (
            x1[:, 128:].reshape(E_CORE, 64, 3).transpose(2, 1, 0)
            .reshape(192, E_CORE))
        fw_t = np.zeros((128, E_PAD), np.float32)
        fw_t[:, :E_CORE] = fw.T
        x2p = np.zeros((E_PAD, 4), np.float32)
        x2p[:E_CORE] = x2

        def rep128(row):
            return np.ascontiguousarray(
                np.broadcast_to(row[None, :], (128, E_PAD)))

        r_v01 = np.empty((128, E_PAD), np.float32)
        r_v01[:64] = x2p[:, 1]
        r_v01[64:] = x2p[:, 2]
        shards.append({
            "x1s_t": np.ascontiguousarray(x1s_t),
            "x1v_t": np.ascontiguousarray(x1v_t),
            "fw_t": np.ascontiguousarray(fw_t),
            "r_s": rep128(x2p[:, 0]),
            "r_v0": rep128(x2p[:, 1]),
            "r_v1": rep128(x2p[:, 2]),
            "r_v2": rep128(x2p[:, 3]),
            "r_v01": r_v01,
        })
    return shards


def run(inputs, trace=False, trace_kwargs=None):
    """Run the kernel; returns (output [E,320] f32, BassKernelResults)."""
    _ensure_repo_on_path()
    from concourse import bass_utils

    nc = _build_nc()
    weights = _fold_weights(inputs)
    shards = _shard_inputs(inputs)
    in_maps = [{**weights, **sh} for sh in shards]

    kwargs = {}
    if trace:
        _install_ntff_hook()
        kwargs.update(trace=True, **(trace_kwargs or {}))
    res = bass_utils.run_bass_kernel_spmd(
        nc, in_maps, core_ids=list(range(N_CORES)), **kwargs)

    out = np.empty((E_FULL, 320), np.float32)
    for c in range(N_CORES):
        o = res.results[c]["out_t"][:, :E_CORE]          # [320, 25000]
        s = slice(c * E_CORE, (c + 1) * E_CORE)
        out[s, :128] = o[:128].T
        out[s, 128:] = (o[128:].reshape(3, 64, E_CORE)
                        .transpose(2, 1, 0).reshape(E_CORE, 192))
    return out, res


def _install_ntff_hook():
    """Shim the missing antenv.axon_hooks so trace=True works under axon."""
    import types
    import antenv
    from concourse import bass_utils
    if "antenv.axon_hooks" in sys.modules:
        return
    mod = types.ModuleType("antenv.axon_hooks")
    _h = [None]
    mod.set_axon_ntff_profile_hook = lambda h: _h.__setitem__(0, h)
    mod.get_axon_ntff_profile_hook = lambda: _h[0]
    sys.modules["antenv.axon_hooks"] = mod
    antenv.axon_hooks = mod
    from trn_agent_boot.trn_boot import _ntff_profile_via_ctypes
    mod.set_axon_ntff_profile_hook(
        _ntff_profile_via_ctypes("/opt/axon/libaxon_pjrt.so"))
    bass_utils.upload_artifacts = lambda tmpdir: tmpdir


def kernel(**inputs) -> np.ndarray:
    out, _ = run(inputs, trace=False)
    return out



# revision 4
# speedup vs baseline: 1.4246x; 1.4246x over previous
"""EquiConv (DeepH-E3) Trainium2 kernel — 8-core data-parallel over edges.

Strategy (channel-major, bf16, 13 matmul slots per 1024-edge supertile):
  - Host folds per-channel weights/constants into matmul weights, casts
    everything to bf16, shards edges across 8 cores, pads to 25*1024 and
    transposes edge tensors to channel-major [C, E].  Per-edge scalars
    (s, v0, v1, v2 from fea_in2) are shipped pre-replicated across
    partitions in bf16; x1s / x1v01 / fw / rep_s / rep_v01 are packed
    into ONE interleaved "bigin" DRAM tensor so each supertile needs
    only 3 load DMAs + 2 store DMAs.
  - Matmul packing: gate and the p2 output are duplicated across both
    PSUM halves via widened weight matrices (idle M side), p3's wc
    matmuls for components 0/1 are merged into one block-diagonal
    matmul, and the vec-path adds are folded into PSUM accumulation by
    seeding the bank with the DVE product (matmul start=False).
  - Elementwise split: GpSimd(Pool) takes the three big SBUF-only
    prescales, ACT takes activations + the wwa/wwb PSUM evacuations,
    DVE takes the PSUM-touching products and bf16 finals (2x mode).
  - Host transposes the bf16 channel-major output back to [E, 320] f32.

Self-contained: hardcodes shapes from the problem spec; no file reads.
"""
import os
import sys

import numpy as np

# ---------------------------------------------------------------- constants
E_FULL = 200000
N_CORES = 8
E_CORE = E_FULL // N_CORES      # 25000
NT = 1024                       # edges per supertile
T_TILES = 25                    # supertiles per core
E_PAD = NT * T_TILES            # 25600
NH = 512                        # psum half
MUL_S = 128
MUL_V = 64

INV_S = 1.0 / np.sqrt(MUL_S)
INV_V = 1.0 / np.sqrt(MUL_V)
SQ2 = 1.0 / np.sqrt(2.0)
SQ3 = 1.0 / np.sqrt(3.0)

BIGIN_W = 5 * NT                # x1s | x1v01 | fw | rep_s | rep_v01
BIGOUT_W = 2 * NT               # out_s | out01

_REPO_CANDIDATES = (
    "/opt/trn_rl_repo",
    "/root/.axon_site/_ro/trn_rl_repo",
)


def _ensure_repo_on_path():
    try:
        import concourse.bass  # noqa: F401
        return
    except ImportError:
        pass
    for p in _REPO_CANDIDATES:
        if os.path.isdir(p) and p not in sys.path:
            sys.path.insert(0, p)
    import concourse.bass  # noqa: F401


_CACHE = {}


def _build_nc():
    """Build + compile the per-core Bass program (cached)."""
    if "nc" in _CACHE:
        return _CACHE["nc"]
    _ensure_repo_on_path()
    import concourse.mybir as mybir
    import concourse.tile as tile
    from concourse import bacc

    F32 = mybir.dt.float32
    BF16 = mybir.dt.bfloat16
    MULT = mybir.AluOpType.mult
    AF = mybir.ActivationFunctionType

    nc = bacc.Bacc(trn_type="TRN2", target_bir_lowering=False, debug=False,
                   num_devices=N_CORES)

    # DRAM inputs (per-core shard) -----------------------------------------
    d_bigin = nc.dram_tensor("bigin", [128, T_TILES * BIGIN_W], BF16,
                             kind="ExternalInput")
    d_x1v2d = nc.dram_tensor("x1v2d", [128, E_PAD], BF16,
                             kind="ExternalInput")
    d_repsv2 = nc.dram_tensor("repsv2", [128, E_PAD], BF16,
                              kind="ExternalInput")
    # folded weights ([K, M] layouts, ready as lhsT)
    d_wa0 = nc.dram_tensor("wa0", [128, 128], BF16, kind="ExternalInput")
    d_wa1d = nc.dram_tensor("wa1d", [128, 128], BF16, kind="ExternalInput")
    d_wb4s = nc.dram_tensor("wb4s", [128, 128], BF16, kind="ExternalInput")
    d_wb5sd = nc.dram_tensor("wb5sd", [128, 128], BF16, kind="ExternalInput")
    d_wb4b = nc.dram_tensor("wb4b", [64, 128], BF16, kind="ExternalInput")
    d_wb5bd = nc.dram_tensor("wb5bd", [64, 128], BF16, kind="ExternalInput")
    d_wp2d = nc.dram_tensor("wp2d", [128, 128], BF16, kind="ExternalInput")
    d_wcd = nc.dram_tensor("wcd", [128, 128], BF16, kind="ExternalInput")
    d_wc = nc.dram_tensor("wc", [64, 64], BF16, kind="ExternalInput")
    d_fc0 = nc.dram_tensor("fc0", [128, 64], BF16, kind="ExternalInput")
    d_fc1 = nc.dram_tensor("fc1", [64, 64], BF16, kind="ExternalInput")
    d_fc2a = nc.dram_tensor("fc2a", [64, 128], BF16, kind="ExternalInput")
    d_fc2bd = nc.dram_tensor("fc2bd", [64, 128], BF16, kind="ExternalInput")
    d_b0 = nc.dram_tensor("b0c", [64, 1], F32, kind="ExternalInput")
    d_b1 = nc.dram_tensor("b1c", [64, 1], F32, kind="ExternalInput")
    d_b2a = nc.dram_tensor("b2a", [128, 1], F32, kind="ExternalInput")
    d_b2bd = nc.dram_tensor("b2bd", [128, 1], F32, kind="ExternalInput")

    d_bigout = nc.dram_tensor("bigout", [128, T_TILES * BIGOUT_W], BF16,
                              kind="ExternalOutput")
    d_out2 = nc.dram_tensor("out2", [64, E_PAD], BF16, kind="ExternalOutput")

    with tile.TileContext(nc) as tc:
        with tc.tile_pool(name="const", bufs=1) as cp, \
             tc.tile_pool(name="io", bufs=3) as io, \
             tc.tile_pool(name="wk", bufs=3) as wk, \
             tc.tile_pool(name="ot", bufs=3) as ot, \
             tc.tile_pool(name="ps", bufs=1, space="PSUM") as ps:

            def const(d, shape, dtype=BF16, lo=0):
                t = cp.tile(shape, dtype, name=d.name + "_sb")
                if lo:
                    nc.sync.dma_start(t[lo:128, :], d.ap())
                else:
                    nc.sync.dma_start(t, d.ap())
                return t

            w_wa0 = const(d_wa0, [128, 128])
            w_wa1d = const(d_wa1d, [128, 128])
            w_wb4s = const(d_wb4s, [128, 128])
            w_wb5sd = const(d_wb5sd, [128, 128])
            w_wp2d = const(d_wp2d, [128, 128])
            w_wcd = const(d_wcd, [128, 128])
            w_fc0 = const(d_fc0, [128, 64])
            w_wc = const(d_wc, [64, 64])
            w_fc1 = const(d_fc1, [64, 64])
            # weights living at partitions 64-127 (rhs at partition offset 64)
            w_wb4b = const(d_wb4b, [128, 128], lo=64)
            w_wb5bd = const(d_wb5bd, [128, 128], lo=64)
            w_fc2a = const(d_fc2a, [128, 128], lo=64)
            w_fc2bd = const(d_fc2bd, [128, 128], lo=64)
            c_b0 = const(d_b0, [64, 1], F32)
            c_b1 = const(d_b1, [128, 1], F32, lo=64)
            c_b2a = const(d_b2a, [128, 1], F32)
            c_b2bd = const(d_b2bd, [128, 1], F32)

            for t in range(T_TILES):
                sl = slice(t * NT, (t + 1) * NT)

                # ---- loads -------------------------------------------
                bigin = io.tile([128, BIGIN_W], BF16)
                nc.sync.dma_start(
                    bigin, d_bigin.ap()[:, t * BIGIN_W:(t + 1) * BIGIN_W])
                x1v2d = io.tile([128, NT], BF16)
                nc.sync.dma_start(x1v2d, d_x1v2d.ap()[:, sl])
                repsv2 = io.tile([128, NT], BF16)
                nc.sync.dma_start(repsv2, d_repsv2.ap()[:, sl])

                x1s = bigin[:, 0 * NT:1 * NT]
                x1v01 = bigin[:, 1 * NT:2 * NT]
                fw = bigin[:, 2 * NT:3 * NT]
                rep_s = bigin[:, 3 * NT:4 * NT]
                rep_v01 = bigin[:, 4 * NT:5 * NT]

                # ---- prescales: Pool (sbuf bf16) ---------------------
                xs_s = wk.tile([128, NT], BF16)
                nc.gpsimd.tensor_tensor(xs_s, x1s, rep_s, MULT)
                xv_p01 = wk.tile([128, NT], BF16)
                nc.gpsimd.tensor_tensor(xv_p01, x1v01, rep_v01, MULT)
                xv_s01 = wk.tile([128, NT], BF16)
                nc.gpsimd.tensor_tensor(xv_s01, x1v01, rep_s, MULT)
                # xvps2 = [x1v2*s (lo) ; x1v2*v2 (hi)]  (DVE, sbuf bf16)
                xvps2 = wk.tile([128, NT], BF16)
                nc.vector.tensor_tensor(xvps2, x1v2d, repsv2, MULT)

                # ---- SBUF result tiles -------------------------------
                h1s = wk.tile([64, NT], BF16)
                h2s = wk.tile([128, NT], BF16)
                sc_silu = wk.tile([128, NT], BF16)
                tg = wk.tile([128, NT], BF16)
                tgp1 = wk.tile([128, NT], BF16)
                wwa_sb = wk.tile([128, NT], BF16)
                wwb_sb = wk.tile([128, NT], BF16)
                sgw2 = wk.tile([128, NT], BF16)
                bigout = ot.tile([128, BIGOUT_W], BF16)
                out2 = ot.tile([128, NT], BF16)

                for h in range(2):
                    hs = slice(h * NH, (h + 1) * NH)

                    a2 = ps.tile([128, NH], F32, tag="a2")
                    scal = ps.tile([128, NH], F32, tag="scal")
                    gate2 = ps.tile([128, NH], F32, tag="gate2")
                    mlp = ps.tile([128, NH], F32, tag="mlp")
                    c01 = ps.tile([128, NH], F32, tag="c01")
                    c2 = ps.tile([128, NH], F32, tag="c2")
                    wwa = ps.tile([128, NH], F32, tag="wwa")
                    wwb2 = ps.tile([128, NH], F32, tag="wwb2")

                    # ---- PE stream (13 matmuls) ----------------------
                    nc.tensor.matmul(a2, w_wp2d, x1s[:, hs],
                                     start=True, stop=True)          # T7
                    nc.tensor.matmul(mlp[0:64, :], w_fc0, fw[:, hs],
                                     start=True, stop=True)          # M1
                    nc.tensor.matmul(scal, w_wa0, xs_s[:, hs],
                                     start=True, stop=False)         # T1
                    nc.tensor.matmul(gate2, w_wa1d, xs_s[:, hs],
                                     start=True, stop=False)         # T4
                    nc.tensor.matmul(scal, w_wb4s, xv_p01[:, hs],
                                     start=False, stop=False)        # T2
                    nc.tensor.matmul(gate2, w_wb5sd, xv_p01[:, hs],
                                     start=False, stop=False)        # T5
                    nc.tensor.matmul(scal, w_wb4b[64:128, :],
                                     xvps2[64:128, hs],
                                     start=False, stop=True,
                                     tile_position=(64, 0))          # T3
                    nc.tensor.matmul(gate2, w_wb5bd[64:128, :],
                                     xvps2[64:128, hs],
                                     start=False, stop=True,
                                     tile_position=(64, 0))          # T6

                    # seed vec banks with the p2 postscale, then let the
                    # wc matmuls accumulate on top (start=False)
                    nc.vector.tensor_tensor(c01, a2, rep_v01[:, hs], MULT)
                    nc.vector.tensor_tensor(c2[64:128, :], a2[64:128, :],
                                            repsv2[64:128, hs], MULT)

                    # ---- radial MLP ----------------------------------
                    nc.scalar.activation(h1s[:, hs], mlp[0:64, :], AF.Silu,
                                         bias=c_b0)                  # A1
                    nc.tensor.matmul(mlp[64:128, :], w_fc1, h1s[:, hs],
                                     start=True, stop=True,
                                     tile_position=(0, 64))          # M2
                    nc.scalar.activation(h2s[64:128, hs], mlp[64:128, :],
                                         AF.Silu,
                                         bias=c_b1[64:128, :])       # A2
                    nc.tensor.matmul(wwa, w_fc2a[64:128, :],
                                     h2s[64:128, hs],
                                     start=True, stop=True,
                                     tile_position=(64, 0))          # M3
                    nc.tensor.matmul(wwb2, w_fc2bd[64:128, :],
                                     h2s[64:128, hs],
                                     start=True, stop=True,
                                     tile_position=(64, 0))          # M4

                    nc.tensor.matmul(c01, w_wcd, xv_s01[:, hs],
                                     start=False, stop=True)         # T8
                    nc.tensor.matmul(c2[64:128, :], w_wc,
                                     xvps2[0:64, hs],
                                     start=False, stop=True,
                                     tile_position=(0, 64))          # T9

                    # ---- gate + e3ElementWise ------------------------
                    nc.scalar.activation(sc_silu[:, hs], scal, AF.Silu)
                    nc.scalar.activation(tg[:, hs], gate2, AF.Tanh,
                                         scale=0.5)
                    # wwb_sb = 0.5*wwb2 + 0.5*b2b  (sigmoid via tanh)
                    nc.scalar.activation(wwb_sb[:, hs], wwb2, AF.Identity,
                                         bias=c_b2bd, scale=0.5)
                    # wwa_sb = wwa + b2a
                    nc.scalar.activation(wwa_sb[:, hs], wwa, AF.Identity,
                                         bias=c_b2a)
                    # sgw2 = (tanh(g/2)+1) * wwb_sb = sigmoid(g)*w_b
                    nc.vector.tensor_scalar_add(tgp1[:, hs], tg[:, hs], 1.0)
                    nc.vector.tensor_tensor(sgw2[:, hs], tgp1[:, hs],
                                            wwb_sb[:, hs], MULT)
                    # out_s = silu(scal) * (wwa + b2a)
                    nc.vector.tensor_tensor(
                        bigout[:, h * NH:(h + 1) * NH], sc_silu[:, hs],
                        wwa_sb[:, hs], MULT)
                    nc.vector.tensor_tensor(
                        bigout[:, NT + h * NH:NT + (h + 1) * NH], c01,
                        sgw2[:, hs], MULT)
                    nc.vector.tensor_tensor(out2[64:128, hs], c2[64:128, :],
                                            sgw2[64:128, hs], MULT)

                # ---- stores ------------------------------------------
                nc.sync.dma_start(
                    d_bigout.ap()[:, t * BIGOUT_W:(t + 1) * BIGOUT_W], bigout)
                nc.sync.dma_start(d_out2.ap()[:, sl], out2[64:128, :])

    nc.compile()
    _CACHE["nc"] = nc
    return nc


def _bf16(x):
    import ml_dtypes
    return np.asarray(x, dtype=np.float32).astype(ml_dtypes.bfloat16)


def _fold_weights(inp):
    """Fold per-channel weights + constants into matmul matrices (bf16)."""
    f = lambda k: np.asarray(inp[k], dtype=np.float32)
    w0f = f("w1_p0") * f("w2_p0")[None, :] * (INV_S * SQ2)
    w1f = f("w1_p1") * f("w2_p1")[None, :] * (INV_S * SQ2)
    w2f = f("w1_p2") * f("w2_p2")[None, :] * (INV_S * SQ2)
    w3f = f("w1_p3") * f("w2_p3")[None, :] * (INV_V * SQ2)
    w4f = f("w1_p4") * f("w2_p4")[None, :] * (INV_V * SQ3 * SQ2)
    w5f = f("w1_p5") * f("w2_p5")[None, :] * (INV_V * SQ3 * SQ2)
    fc2 = f("fc_w2")
    b2 = f("fc_b2")
    wcd = np.zeros((128, 128), np.float32)
    wcd[:64, :64] = w3f
    wcd[64:, 64:] = w3f
    c = np.ascontiguousarray
    return {
        "wa0": _bf16(w0f),
        "wa1d": _bf16(np.concatenate([w1f, w1f], axis=1)),
        "wb4s": _bf16(np.concatenate([w4f, w4f], axis=0)),
        "wb5sd": _bf16(np.tile(w5f, (2, 2))),
        "wb4b": _bf16(w4f),
        "wb5bd": _bf16(np.concatenate([w5f, w5f], axis=1)),
        "wp2d": _bf16(np.concatenate([w2f, w2f], axis=1)),
        "wcd": _bf16(wcd),
        "wc": _bf16(w3f),
        "fc0": _bf16(f("fc_w0")),
        "fc1": _bf16(f("fc_w1")),
        "fc2a": _bf16(fc2[:, :128]),
        "fc2bd": _bf16(np.concatenate([fc2[:, 128:], fc2[:, 128:]], axis=1)),
        "b0c": c(f("fc_b0")[:, None]),
        "b1c": c(f("fc_b1")[:, None]),
        "b2a": c(b2[:128, None]),
        "b2bd": c(0.5 * np.concatenate([b2[128:], b2[128:]])[:, None]),
    }


def _shard_inputs(inp):
    """Per-core bf16 channel-major shards (padded to E_PAD edges)."""
    import ml_dtypes
    BF = ml_dtypes.bfloat16
    fea_in1 = np.asarray(inp["fea_in1"], dtype=np.float32)
    fea_in2 = np.asarray(inp["fea_in2"], dtype=np.float32)
    fea_w = np.asarray(inp["fea_weight"], dtype=np.float32)
    shards = []
    for c in range(N_CORES):
        s = slice(c * E_CORE, (c + 1) * E_CORE)
        x1 = fea_in1[s]
        x2 = fea_in2[s]
        fwm = fea_w[s]

        x1s_t = np.zeros((128, E_PAD), BF)
        x1s_t[:, :E_CORE] = x1[:, :128].T
        x1v = x1[:, 128:].reshape(E_CORE, 64, 3)
        x1v01_t = np.zeros((128, E_PAD), BF)
        x1v01_t[:64, :E_CORE] = x1v[:, :, 0].T
        x1v01_t[64:, :E_CORE] = x1v[:, :, 1].T
        x1v2d_t = np.zeros((128, E_PAD), BF)
        x1v2d_t[:64, :E_CORE] = x1v[:, :, 2].T
        x1v2d_t[64:, :E_CORE] = x1v[:, :, 2].T
        fw_t = np.zeros((128, E_PAD), BF)
        fw_t[:, :E_CORE] = fwm.T
        x2p = np.zeros((E_PAD, 4), np.float32)
        x2p[:E_CORE] = x2

        rep_s = np.broadcast_to(
            x2p[:, 0].astype(BF)[None, :], (128, E_PAD))
        rep_v01 = np.empty((128, E_PAD), BF)
        rep_v01[:64] = x2p[:, 1].astype(BF)
        rep_v01[64:] = x2p[:, 2].astype(BF)
        repsv2 = np.empty((128, E_PAD), BF)
        repsv2[:64] = x2p[:, 0].astype(BF)
        repsv2[64:] = x2p[:, 3].astype(BF)

        # interleave into bigin: [128, T, 5, NT]
        big = np.stack([
            x1s_t.reshape(128, T_TILES, NT),
            x1v01_t.reshape(128, T_TILES, NT),
            fw_t.reshape(128, T_TILES, NT),
            np.ascontiguousarray(rep_s).reshape(128, T_TILES, NT),
            rep_v01.reshape(128, T_TILES, NT),
        ], axis=2)                                  # [128, T, 5, NT]
        shards.append({
            "bigin": np.ascontiguousarray(
                big.reshape(128, T_TILES * BIGIN_W)),
            "x1v2d": x1v2d_t,
            "repsv2": repsv2,
        })
    return shards


def run(inputs, trace=False, trace_kwargs=None):
    """Run the kernel; returns (output [E,320] f32, BassKernelResults)."""
    _ensure_repo_on_path()
    from concourse import bass_utils

    nc = _build_nc()
    weights = _fold_weights(inputs)
    shards = _shard_inputs(inputs)
    in_maps = [{**weights, **sh} for sh in shards]

    kwargs = {}
    if trace:
        _install_ntff_hook()
        kwargs.update(trace=True, **(trace_kwargs or {}))
    res = bass_utils.run_bass_kernel_spmd(
        nc, in_maps, core_ids=list(range(N_CORES)), **kwargs)

    out = np.empty((E_FULL, 320), np.float32)
    for c in range(N_CORES):
        bo = np.asarray(res.results[c]["bigout"], dtype=np.float32)
        o2 = np.asarray(res.results[c]["out2"], dtype=np.float32)
        bo = bo.reshape(128, T_TILES, 2, NT)
        out_s = bo[:, :, 0, :].reshape(128, E_PAD)[:, :E_CORE]
        out01 = bo[:, :, 1, :].reshape(128, E_PAD)[:, :E_CORE]
        s = slice(c * E_CORE, (c + 1) * E_CORE)
        out[s, :128] = out_s.T
        # vec layout: out[e, 128 + u*3 + i]
        vec = np.empty((E_CORE, 64, 3), np.float32)
        vec[:, :, 0] = out01[:64].T
        vec[:, :, 1] = out01[64:].T
        vec[:, :, 2] = o2[:, :E_CORE].T
        out[s, 128:] = vec.reshape(E_CORE, 192)
    return out, res


def _install_ntff_hook():
    """Shim the missing antenv.axon_hooks so trace=True works under axon."""
    import types
    import antenv
    from concourse import bass_utils
    if "antenv.axon_hooks" in sys.modules:
        return
    mod = types.ModuleType("antenv.axon_hooks")
    _h = [None]
    mod.set_axon_ntff_profile_hook = lambda h: _h.__setitem__(0, h)
    mod.get_axon_ntff_profile_hook = lambda: _h[0]
    sys.modules["antenv.axon_hooks"] = mod
    antenv.axon_hooks = mod
    from trn_agent_boot.trn_boot import _ntff_profile_via_ctypes
    mod.set_axon_ntff_profile_hook(
        _ntff_profile_via_ctypes("/opt/axon/libaxon_pjrt.so"))
    bass_utils.upload_artifacts = lambda tmpdir: tmpdir


def kernel(**inputs) -> np.ndarray:
    out, _ = run(inputs, trace=False)
    return out


# revision 12
# speedup vs baseline: 1.4324x; 1.0055x over previous
"""EquiConv (DeepH-E3) Trainium2 kernel — 8-core data-parallel over edges.

Strategy (channel-major, bf16, 13 matmul slots per 1024-edge supertile):
  - Host folds per-channel weights/constants into matmul weights, casts
    everything to bf16, shards edges across 8 cores, pads to 25*1024 and
    transposes edge tensors to channel-major [C, E].  Per-edge scalars
    (s, v0, v1, v2 from fea_in2) are shipped pre-replicated across
    partitions in bf16; x1s / x1v01 / fw / rep_s / rep_v01 are packed
    into ONE interleaved "bigin" DRAM tensor so each supertile needs
    only 3 load DMAs + 2 store DMAs.
  - Matmul packing: gate and the p2 output are duplicated across both
    PSUM halves via widened weight matrices (idle M side), p3's wc
    matmuls for components 0/1 are merged into one block-diagonal
    matmul, and the vec-path adds are folded into PSUM accumulation by
    seeding the bank with the DVE product (matmul start=False).
  - Elementwise split: GpSimd(Pool) takes the three big SBUF-only
    prescales, ACT takes activations + the wwa/wwb PSUM evacuations,
    DVE takes the PSUM-touching products and bf16 finals (2x mode).
  - Host transposes the bf16 channel-major output back to [E, 320] f32.

Self-contained: hardcodes shapes from the problem spec; no file reads.
"""
import os
import sys

import numpy as np

# ---------------------------------------------------------------- constants
E_FULL = 200000
N_CORES = 8
E_CORE = E_FULL // N_CORES      # 25000
NT = 1024                       # edges per supertile
T_TILES = 25                    # supertiles per core
E_PAD = NT * T_TILES            # 25600
NH = 512                        # psum half
MUL_S = 128
MUL_V = 64

INV_S = 1.0 / np.sqrt(MUL_S)
INV_V = 1.0 / np.sqrt(MUL_V)
SQ2 = 1.0 / np.sqrt(2.0)
SQ3 = 1.0 / np.sqrt(3.0)

BIGIN_W = 5 * NT                # x1s | x1v01 | fw | rep_s | rep_v01
BIGOUT_W = 2 * NT               # out_s | out01

_REPO_CANDIDATES = (
    "/opt/trn_rl_repo",
    "/root/.axon_site/_ro/trn_rl_repo",
)


def _ensure_repo_on_path():
    try:
        import concourse.bass  # noqa: F401
        return
    except ImportError:
        pass
    for p in _REPO_CANDIDATES:
        if os.path.isdir(p) and p not in sys.path:
            sys.path.insert(0, p)
    import concourse.bass  # noqa: F401


_CACHE = {}


def _build_nc():
    """Build + compile the per-core Bass program (cached)."""
    if "nc" in _CACHE:
        return _CACHE["nc"]
    _ensure_repo_on_path()
    import concourse.mybir as mybir
    import concourse.tile as tile
    from concourse import bacc

    F32 = mybir.dt.float32
    BF16 = mybir.dt.bfloat16
    MULT = mybir.AluOpType.mult
    ADD = mybir.AluOpType.add
    AF = mybir.ActivationFunctionType

    nc = bacc.Bacc(trn_type="TRN2", target_bir_lowering=False, debug=False,
                   num_devices=N_CORES)

    # DRAM inputs (per-core shard) -----------------------------------------
    d_bigin = nc.dram_tensor("bigin", [128, T_TILES * BIGIN_W], BF16,
                             kind="ExternalInput")
    d_x1v2d = nc.dram_tensor("x1v2d", [128, E_PAD], BF16,
                             kind="ExternalInput")
    d_repsv2 = nc.dram_tensor("repsv2", [128, E_PAD], BF16,
                              kind="ExternalInput")
    # folded weights ([K, M] layouts, ready as lhsT)
    d_wa0 = nc.dram_tensor("wa0", [128, 128], BF16, kind="ExternalInput")
    d_wa1d = nc.dram_tensor("wa1d", [128, 128], BF16, kind="ExternalInput")
    d_wb4s = nc.dram_tensor("wb4s", [128, 128], BF16, kind="ExternalInput")
    d_wb5sd = nc.dram_tensor("wb5sd", [128, 128], BF16, kind="ExternalInput")
    d_wb4b = nc.dram_tensor("wb4b", [64, 128], BF16, kind="ExternalInput")
    d_wb5bd = nc.dram_tensor("wb5bd", [64, 128], BF16, kind="ExternalInput")
    d_wp2d = nc.dram_tensor("wp2d", [128, 128], BF16, kind="ExternalInput")
    d_wcd = nc.dram_tensor("wcd", [128, 128], BF16, kind="ExternalInput")
    d_wc = nc.dram_tensor("wc", [64, 64], BF16, kind="ExternalInput")
    d_fc0 = nc.dram_tensor("fc0", [128, 64], BF16, kind="ExternalInput")
    d_fc1 = nc.dram_tensor("fc1", [64, 64], BF16, kind="ExternalInput")
    d_fc2a = nc.dram_tensor("fc2a", [64, 128], BF16, kind="ExternalInput")
    d_fc2bd = nc.dram_tensor("fc2bd", [64, 128], BF16, kind="ExternalInput")
    d_b0 = nc.dram_tensor("b0c", [64, 1], F32, kind="ExternalInput")
    d_b1 = nc.dram_tensor("b1c", [64, 1], F32, kind="ExternalInput")
    d_b2a = nc.dram_tensor("b2a", [128, 1], F32, kind="ExternalInput")
    d_b2bd = nc.dram_tensor("b2bd", [128, 1], F32, kind="ExternalInput")

    d_bigout = nc.dram_tensor("bigout", [128, T_TILES * BIGOUT_W], BF16,
                              kind="ExternalOutput")
    d_out2 = nc.dram_tensor("out2", [64, E_PAD], BF16, kind="ExternalOutput")

    with tile.TileContext(nc) as tc:
        with tc.tile_pool(name="const", bufs=1) as cp, \
             tc.tile_pool(name="io", bufs=3) as io, \
             tc.tile_pool(name="wk", bufs=3) as wk, \
             tc.tile_pool(name="ot", bufs=3) as ot, \
             tc.tile_pool(name="ps", bufs=1, space="PSUM") as ps:

            def const(d, shape, dtype=BF16, lo=0):
                t = cp.tile(shape, dtype, name=d.name + "_sb")
                if lo:
                    nc.sync.dma_start(t[lo:128, :], d.ap())
                else:
                    nc.sync.dma_start(t, d.ap())
                return t

            w_wa0 = const(d_wa0, [128, 128])
            w_wa1d = const(d_wa1d, [128, 128])
            w_wb4s = const(d_wb4s, [128, 128])
            w_wb5sd = const(d_wb5sd, [128, 128])
            w_wp2d = const(d_wp2d, [128, 128])
            w_wcd = const(d_wcd, [128, 128])
            w_fc0 = const(d_fc0, [128, 64])
            w_wc = const(d_wc, [64, 64])
            w_fc1 = const(d_fc1, [64, 64])
            # weights living at partitions 64-127 (rhs at partition offset 64)
            w_wb4b = const(d_wb4b, [128, 128], lo=64)
            w_wb5bd = const(d_wb5bd, [128, 128], lo=64)
            w_fc2a = const(d_fc2a, [128, 128], lo=64)
            w_fc2bd = const(d_fc2bd, [128, 128], lo=64)
            c_b0 = const(d_b0, [64, 1], F32)
            c_b1 = const(d_b1, [128, 1], F32, lo=64)
            c_b2a = const(d_b2a, [128, 1], F32)
            c_b2bd = const(d_b2bd, [128, 1], F32)

            for t in range(T_TILES):
                sl = slice(t * NT, (t + 1) * NT)

                # ---- loads -------------------------------------------
                bigin = io.tile([128, BIGIN_W], BF16)
                nc.sync.dma_start(
                    bigin, d_bigin.ap()[:, t * BIGIN_W:(t + 1) * BIGIN_W])
                x1v2d = io.tile([128, NT], BF16)
                nc.sync.dma_start(x1v2d, d_x1v2d.ap()[:, sl])
                repsv2 = io.tile([128, NT], BF16)
                nc.sync.dma_start(repsv2, d_repsv2.ap()[:, sl])

                x1s = bigin[:, 0 * NT:1 * NT]
                x1v01 = bigin[:, 1 * NT:2 * NT]
                fw = bigin[:, 2 * NT:3 * NT]
                rep_s = bigin[:, 3 * NT:4 * NT]
                rep_v01 = bigin[:, 4 * NT:5 * NT]

                # ---- prescales: Pool (sbuf bf16) ---------------------
                xs_s = wk.tile([128, NT], BF16)
                nc.gpsimd.tensor_tensor(xs_s, x1s, rep_s, MULT)
                xv_p01 = wk.tile([128, NT], BF16)
                nc.gpsimd.tensor_tensor(xv_p01, x1v01, rep_v01, MULT)
                xv_s01 = wk.tile([128, NT], BF16)
                nc.gpsimd.tensor_tensor(xv_s01, x1v01, rep_s, MULT)
                # xvps2 = [x1v2*s (lo) ; x1v2*v2 (hi)]  (DVE, sbuf bf16)
                xvps2 = wk.tile([128, NT], BF16)
                nc.vector.tensor_tensor(xvps2, x1v2d, repsv2, MULT)

                # ---- SBUF result tiles -------------------------------
                h1s = wk.tile([64, NT], BF16)
                h2s = wk.tile([128, NT], BF16)
                sc_silu = wk.tile([128, NT], BF16)
                tg = wk.tile([128, NT], BF16)
                wwa_sb = wk.tile([128, NT], BF16)
                wwb_sb = wk.tile([128, NT], BF16)
                sgw2 = wk.tile([128, NT], BF16)
                bigout = ot.tile([128, BIGOUT_W], BF16)
                out2 = ot.tile([128, NT], BF16)

                for h in range(2):
                    hs = slice(h * NH, (h + 1) * NH)

                    a2 = ps.tile([128, NH], F32, tag="a2")
                    scal = ps.tile([128, NH], F32, tag="scal")
                    gate2 = ps.tile([128, NH], F32, tag="gate2")
                    mlp = ps.tile([128, NH], F32, tag="mlp")
                    c01 = ps.tile([128, NH], F32, tag="c01")
                    c2 = ps.tile([128, NH], F32, tag="c2")
                    wwa = ps.tile([128, NH], F32, tag="wwa")
                    wwb2 = ps.tile([128, NH], F32, tag="wwb2")

                    # ---- PE stream (13 matmuls, readiness order) -----
                    nc.tensor.matmul(a2, w_wp2d, x1s[:, hs],
                                     start=True, stop=True)          # T7
                    nc.tensor.matmul(mlp[0:64, :], w_fc0, fw[:, hs],
                                     start=True, stop=True)          # M1
                    nc.tensor.matmul(scal, w_wb4b[64:128, :],
                                     xvps2[64:128, hs],
                                     start=True, stop=False,
                                     tile_position=(64, 0))          # T3
                    nc.tensor.matmul(gate2, w_wb5bd[64:128, :],
                                     xvps2[64:128, hs],
                                     start=True, stop=False,
                                     tile_position=(64, 0))          # T6
                    nc.tensor.matmul(scal, w_wa0, xs_s[:, hs],
                                     start=False, stop=False)        # T1
                    nc.tensor.matmul(gate2, w_wa1d, xs_s[:, hs],
                                     start=False, stop=False)        # T4
                    nc.tensor.matmul(scal, w_wb4s, xv_p01[:, hs],
                                     start=False, stop=True)         # T2
                    nc.tensor.matmul(gate2, w_wb5sd, xv_p01[:, hs],
                                     start=False, stop=True)         # T5

                    # seed vec banks with the p2 postscale, then let the
                    # wc matmuls accumulate on top (start=False)
                    nc.vector.tensor_tensor(c01, a2, rep_v01[:, hs], MULT)
                    nc.vector.tensor_tensor(c2[64:128, :], a2[64:128, :],
                                            repsv2[64:128, hs], MULT)

                    # ---- radial MLP ----------------------------------
                    nc.scalar.activation(h1s[:, hs], mlp[0:64, :], AF.Silu,
                                         bias=c_b0)                  # A1
                    nc.tensor.matmul(mlp[64:128, :], w_fc1, h1s[:, hs],
                                     start=True, stop=True,
                                     tile_position=(0, 64))          # M2
                    nc.scalar.activation(h2s[64:128, hs], mlp[64:128, :],
                                         AF.Silu,
                                         bias=c_b1[64:128, :])       # A2
                    nc.tensor.matmul(wwa, w_fc2a[64:128, :],
                                     h2s[64:128, hs],
                                     start=True, stop=True,
                                     tile_position=(64, 0))          # M3
                    nc.tensor.matmul(wwb2, w_fc2bd[64:128, :],
                                     h2s[64:128, hs],
                                     start=True, stop=True,
                                     tile_position=(64, 0))          # M4

                    nc.tensor.matmul(c01, w_wcd, xv_s01[:, hs],
                                     start=False, stop=True)         # T8
                    nc.tensor.matmul(c2[64:128, :], w_wc,
                                     xvps2[0:64, hs],
                                     start=False, stop=True,
                                     tile_position=(0, 64))          # T9

                    # ---- gate + e3ElementWise ------------------------
                    nc.scalar.activation(sc_silu[:, hs], scal, AF.Silu)
                    nc.scalar.activation(tg[:, hs], gate2, AF.Tanh,
                                         scale=0.5)
                    # wwa_sb = wwa + b2a
                    nc.scalar.activation(wwa_sb[:, hs], wwa, AF.Identity,
                                         bias=c_b2a)
                    # wwb_sb = 0.5*wwb2 + 0.5*b2b  (sigmoid via tanh)
                    nc.scalar.activation(wwb_sb[:, hs], wwb2, AF.Identity,
                                         bias=c_b2bd, scale=0.5)
                    # sgw2 = (tanh(g/2)+1) * wwb_sb = sigmoid(g)*w_b
                    nc.vector.scalar_tensor_tensor(sgw2[:, hs], tg[:, hs],
                                                   1.0, wwb_sb[:, hs],
                                                   ADD, MULT)
                    # out_s = silu(scal) * (wwa + b2a)
                    nc.vector.tensor_tensor(
                        bigout[:, h * NH:(h + 1) * NH], sc_silu[:, hs],
                        wwa_sb[:, hs], MULT)
                    nc.vector.tensor_tensor(
                        bigout[:, NT + h * NH:NT + (h + 1) * NH], c01,
                        sgw2[:, hs], MULT)
                    nc.vector.tensor_tensor(out2[64:128, hs], c2[64:128, :],
                                            sgw2[64:128, hs], MULT)

                # ---- stores ------------------------------------------
                nc.sync.dma_start(
                    d_bigout.ap()[:, t * BIGOUT_W:(t + 1) * BIGOUT_W], bigout)
                nc.sync.dma_start(d_out2.ap()[:, sl], out2[64:128, :])

    nc.compile()
    _CACHE["nc"] = nc
    return nc


def _bf16(x):
    import ml_dtypes
    return np.asarray(x, dtype=np.float32).astype(ml_dtypes.bfloat16)


def _fold_weights(inp):
    """Fold per-channel weights + constants into matmul matrices (bf16)."""
    f = lambda k: np.asarray(inp[k], dtype=np.float32)
    w0f = f("w1_p0") * f("w2_p0")[None, :] * (INV_S * SQ2)
    w1f = f("w1_p1") * f("w2_p1")[None, :] * (INV_S * SQ2)
    w2f = f("w1_p2") * f("w2_p2")[None, :] * (INV_S * SQ2)
    w3f = f("w1_p3") * f("w2_p3")[None, :] * (INV_V * SQ2)
    w4f = f("w1_p4") * f("w2_p4")[None, :] * (INV_V * SQ3 * SQ2)
    w5f = f("w1_p5") * f("w2_p5")[None, :] * (INV_V * SQ3 * SQ2)
    fc2 = f("fc_w2")
    b2 = f("fc_b2")
    wcd = np.zeros((128, 128), np.float32)
    wcd[:64, :64] = w3f
    wcd[64:, 64:] = w3f
    c = np.ascontiguousarray
    return {
        "wa0": _bf16(w0f),
        "wa1d": _bf16(np.concatenate([w1f, w1f], axis=1)),
        "wb4s": _bf16(np.concatenate([w4f, w4f], axis=0)),
        "wb5sd": _bf16(np.tile(w5f, (2, 2))),
        "wb4b": _bf16(w4f),
        "wb5bd": _bf16(np.concatenate([w5f, w5f], axis=1)),
        "wp2d": _bf16(np.concatenate([w2f, w2f], axis=1)),
        "wcd": _bf16(wcd),
        "wc": _bf16(w3f),
        "fc0": _bf16(f("fc_w0")),
        "fc1": _bf16(f("fc_w1")),
        "fc2a": _bf16(fc2[:, :128]),
        "fc2bd": _bf16(np.concatenate([fc2[:, 128:], fc2[:, 128:]], axis=1)),
        "b0c": c(f("fc_b0")[:, None]),
        "b1c": c(f("fc_b1")[:, None]),
        "b2a": c(b2[:128, None]),
        "b2bd": c(0.5 * np.concatenate([b2[128:], b2[128:]])[:, None]),
    }


def _shard_inputs(inp):
    """Per-core bf16 channel-major shards (padded to E_PAD edges)."""
    import ml_dtypes
    BF = ml_dtypes.bfloat16
    fea_in1 = np.asarray(inp["fea_in1"], dtype=np.float32)
    fea_in2 = np.asarray(inp["fea_in2"], dtype=np.float32)
    fea_w = np.asarray(inp["fea_weight"], dtype=np.float32)
    shards = []
    for c in range(N_CORES):
        s = slice(c * E_CORE, (c + 1) * E_CORE)
        x1 = fea_in1[s]
        x2 = fea_in2[s]
        fwm = fea_w[s]

        x1s_t = np.zeros((128, E_PAD), BF)
        x1s_t[:, :E_CORE] = x1[:, :128].T
        x1v = x1[:, 128:].reshape(E_CORE, 64, 3)
        x1v01_t = np.zeros((128, E_PAD), BF)
        x1v01_t[:64, :E_CORE] = x1v[:, :, 0].T
        x1v01_t[64:, :E_CORE] = x1v[:, :, 1].T
        x1v2d_t = np.zeros((128, E_PAD), BF)
        x1v2d_t[:64, :E_CORE] = x1v[:, :, 2].T
        x1v2d_t[64:, :E_CORE] = x1v[:, :, 2].T
        fw_t = np.zeros((128, E_PAD), BF)
        fw_t[:, :E_CORE] = fwm.T
        x2p = np.zeros((E_PAD, 4), np.float32)
        x2p[:E_CORE] = x2

        rep_s = np.broadcast_to(
            x2p[:, 0].astype(BF)[None, :], (128, E_PAD))
        rep_v01 = np.empty((128, E_PAD), BF)
        rep_v01[:64] = x2p[:, 1].astype(BF)
        rep_v01[64:] = x2p[:, 2].astype(BF)
        repsv2 = np.empty((128, E_PAD), BF)
        repsv2[:64] = x2p[:, 0].astype(BF)
        repsv2[64:] = x2p[:, 3].astype(BF)

        # interleave into bigin: [128, T, 5, NT]
        big = np.stack([
            x1s_t.reshape(128, T_TILES, NT),
            x1v01_t.reshape(128, T_TILES, NT),
            fw_t.reshape(128, T_TILES, NT),
            np.ascontiguousarray(rep_s).reshape(128, T_TILES, NT),
            rep_v01.reshape(128, T_TILES, NT),
        ], axis=2)                                  # [128, T, 5, NT]
        shards.append({
            "bigin": np.ascontiguousarray(
                big.reshape(128, T_TILES * BIGIN_W)),
            "x1v2d": x1v2d_t,
            "repsv2": repsv2,
        })
    return shards


def run(inputs, trace=False, trace_kwargs=None):
    """Run the kernel; returns (output [E,320] f32, BassKernelResults)."""
    _ensure_repo_on_path()
    from concourse import bass_utils

    nc = _build_nc()
    weights = _fold_weights(inputs)
    shards = _shard_inputs(inputs)
    in_maps = [{**weights, **sh} for sh in shards]

    kwargs = {}
    if trace:
        _install_ntff_hook()
        kwargs.update(trace=True, **(trace_kwargs or {}))
    res = bass_utils.run_bass_kernel_spmd(
        nc, in_maps, core_ids=list(range(N_CORES)), **kwargs)

    out = np.empty((E_FULL, 320), np.float32)
    for c in range(N_CORES):
        bo = np.asarray(res.results[c]["bigout"], dtype=np.float32)
        o2 = np.asarray(res.results[c]["out2"], dtype=np.float32)
        bo = bo.reshape(128, T_TILES, 2, NT)
        out_s = bo[:, :, 0, :].reshape(128, E_PAD)[:, :E_CORE]
        out01 = bo[:, :, 1, :].reshape(128, E_PAD)[:, :E_CORE]
        s = slice(c * E_CORE, (c + 1) * E_CORE)
        out[s, :128] = out_s.T
        # vec layout: out[e, 128 + u*3 + i]
        vec = np.empty((E_CORE, 64, 3), np.float32)
        vec[:, :, 0] = out01[:64].T
        vec[:, :, 1] = out01[64:].T
        vec[:, :, 2] = o2[:, :E_CORE].T
        out[s, 128:] = vec.reshape(E_CORE, 192)
    return out, res


def _install_ntff_hook():
    """Shim the missing antenv.axon_hooks so trace=True works under axon."""
    import types
    import antenv
    from concourse import bass_utils
    if "antenv.axon_hooks" in sys.modules:
        return
    mod = types.ModuleType("antenv.axon_hooks")
    _h = [None]
    mod.set_axon_ntff_profile_hook = lambda h: _h.__setitem__(0, h)
    mod.get_axon_ntff_profile_hook = lambda: _h[0]
    sys.modules["antenv.axon_hooks"] = mod
    antenv.axon_hooks = mod
    from trn_agent_boot.trn_boot import _ntff_profile_via_ctypes
    mod.set_axon_ntff_profile_hook(
        _ntff_profile_via_ctypes("/opt/axon/libaxon_pjrt.so"))
    bass_utils.upload_artifacts = lambda tmpdir: tmpdir


def kernel(**inputs) -> np.ndarray:
    out, _ = run(inputs, trace=False)
    return out
